# revision 30
# baseline (speedup 1.0000x reference)
"""Trainium2 Bass kernel: batched ChebConv GNN with L1-distance adjacency.

Pipeline per sample (N=512 nodes, F=625 features):
  1. Sort nodes by attention (host). All pairs with |att_i-att_j| <= 0.05
     then lie within a rank band |i-j| <= w (w computed exactly on host).
  2. Banded pairwise L1 distances on device using the exact identity
     sum_f |a-b| = 2*sum_f max(a,b) - S_i - S_j  (S = row sums). The DVE
     max ops are parity-split (odd offsets read a one-element-shifted
     copy of the features) so every instruction is 4B-aligned and runs
     in packed 2-elem/cycle fp16 mode; a PE staircase-matmul does the
     cross-partition feature reduction.
  3. Threshold masks -> banded adjacency strips -> PE-transposed into
     per-node contiguous runs -> scattered to a DRAM matrix (one merged
     DMA per triangle, ~100-byte descriptor per node; identity diagonal
     folded in as a ones column; lower triangle via a skewed DRAM
     reload + reversed anti-identity PE transpose). Degrees come
     straight from column-sums of the strips (2 small PE matmuls), so
     the normalization never waits on the scattered matrix.
  4. Degree-normalized ChebConv x2 as fp16 PE matmuls in transposed
     layouts (fp32 PSUM accumulation).
DMAs are merged into few large multi-dim-AP transfers (the per-exec
critical path here is DMA count x fixed latency, not bandwidth).
All heavy data is fp16 on device (PSUM accumulation and thresholding
math stay fp32; fp16 rounding perturbs the L1 distance by ~4e-3 against
a threshold of 180, flipping ~1e-4 of edge decisions).
Data parallel over batch: 16 samples, 8 cores, 2 samples/core.
"""

import numpy as np
from contextlib import ExitStack

B, N = 16, 512
F, FH = 625, 937
FHP = 1024  # FH padded to 8x128 for merged weight/bias loads
FCH, NFCH = 125, 5  # feature chunks: 5 x 125 = 625
NCORES = 8
SPB = B // NCORES  # samples per core
DIST_THRESH, ATT_THRESH = 180.0, 0.05
DCH = 48  # band offsets per PSUM group (psM tile base partition stays 0)

# FH row blocks (7x128 + 41)
FH_BLOCKS = [(o, min(128, FH - o)) for o in range(0, FH, 128)]
NKB = len(FH_BLOCKS)

_prog_cache = {}


def _build_program(w, reps=1, ablate=()):
    """Build the SPMD Bass program for band half-width w. Returns (nc, WROW).

    ablate: dev-only timing switches that skip named phases (results become
    garbage): band_dve, scatter, cheb, setup.
    """
    import concourse.bass as bass
    import concourse.bacc as bacc
    import concourse.mybir as mybir
    import concourse.tile as tile

    dt = mybir.dt
    fp = dt.float32
    mdt = dt.float16
    AF = mybir.ActivationFunctionType
    OP = mybir.AluOpType
    AP = bass.AP

    padw = ((w + 8 + 7) // 8) * 8  # row pad; >= w+8 so lower-run wraps stay in pad
    WROW = N + padw  # padded row width for xpt/attp/adjacency rows
    ABASE = padw + 16  # guard before adjacency row 0 (lower-run wrap at k=0)
    TBASE = padw + 16  # guard before strip row 0 (skewed reload of group d0=1)

    nc = bacc.Bacc()
    xpt_p = nc.declare_dram_parameter("xpt", [SPB, F, WROW], mdt, isOutput=False)
    xp_p = nc.declare_dram_parameter("xp", [SPB, N, F], mdt, isOutput=False)
    attp_p = nc.declare_dram_parameter("attp", [SPB, WROW], fp, isOutput=False)
    w1_p = nc.declare_dram_parameter("w1", [2, F, FH], mdt, isOutput=False)
    b1_p = nc.declare_dram_parameter("b1", [FHP], fp, isOutput=False)
    w2_p = nc.declare_dram_parameter("w2", [2, FHP, F], mdt, isOutput=False)
    b2_p = nc.declare_dram_parameter("b2", [F + 15], fp, isOutput=False)
    out_p = nc.declare_dram_parameter("outT", [SPB, F, N], mdt, isOutput=True)
    ones_p = nc.declare_dram_parameter("c_ones", [128, 1], mdt, isOutput=False)
    onesr_p = nc.declare_dram_parameter("c_onesr", [1, 128], mdt, isOutput=False)
    estep_p = nc.declare_dram_parameter("c_estep", [FCH, 95], mdt, isOutput=False)
    ident_p = nc.declare_dram_parameter("c_ident", [128, 128], mdt, isOutput=False)
    # anti-identity (reversal) matrices, one per distinct band-group size
    dn_sizes = sorted({min(DCH, w - d0 + 1) for d0 in range(1, w + 1, DCH)})
    anti_p = nc.declare_dram_parameter("c_anti", [len(dn_sizes), DCH, DCH], mdt,
                                       isOutput=False)
    zeros_p = nc.declare_dram_parameter("c_zeros", [128, WROW], mdt, isOutput=False)

    # internal DRAM scratch, one set per sample slot
    a_scr = [nc.dram_tensor(f"a_scr{b}", [ABASE + 512 * WROW + WROW], mdt)
             for b in range(SPB)]
    s_scr = [nc.dram_tensor(f"s_scr{b}", [WROW], fp) for b in range(SPB)]
    t_scr = [nc.dram_tensor(f"t_scr{b}", [TBASE + w + DCH * N], mdt)
             for b in range(SPB)]

    with tile.TileContext(nc) as tc, ExitStack() as ctx:
        cst = ctx.enter_context(tc.tile_pool(name="cst", bufs=1))
        xtp = ctx.enter_context(tc.tile_pool(name="xtp", bufs=2))
        xpp = ctx.enter_context(tc.tile_pool(name="xpp", bufs=1))
        mxp = ctx.enter_context(tc.tile_pool(name="mxp", bufs=2))
        bnd = ctx.enter_context(tc.tile_pool(name="bnd", bufs=1))
        amp = ctx.enter_context(tc.tile_pool(name="amp", bufs=1))
        acp = ctx.enter_context(tc.tile_pool(name="acp", bufs=1))
        wsp = ctx.enter_context(tc.tile_pool(name="wsp", bufs=1))
        otp = ctx.enter_context(tc.tile_pool(name="otp", bufs=1))
        psp = ctx.enter_context(tc.tile_pool(name="psp", bufs=3, space="PSUM"))
        psb = ctx.enter_context(tc.tile_pool(name="psb", bufs=1, space="PSUM"))
        pst = ctx.enter_context(tc.tile_pool(name="pst", bufs=2, space="PSUM"))

        ones = cst.tile([128, 1], mdt, tag="ones")
        nc.sync.dma_start(ones[:], ones_p[:, :])
        onesr = cst.tile([1, 128], mdt, tag="onesr")
        nc.sync.dma_start(onesr[:], onesr_p[:, :])
        ident = cst.tile([128, 128], mdt, tag="ident")
        nc.sync.dma_start(ident[:], ident_p[:, :])
        anti = {}
        for si, dsz in enumerate(dn_sizes):
            anti[dsz] = cst.tile([DCH, DCH], mdt, tag=f"anti{si}",
                                 name=f"anti{si}")
            nc.sync.dma_start(anti[dsz][:], anti_p[si, :, :])
        # staircase selector: estep[:, 47-di : 47-di+dn] is a [FCH, dn]
        # matrix whose only nonzero column is column di (all ones) -> matmul
        # with it as lhsT reduces partitions into PSUM row di (base 0)
        estep = cst.tile([FCH, 95], mdt, tag="estep")
        nc.sync.dma_start(estep[:], estep_p[:, :])

        BD = 16  # band offsets per (even, odd) DVE instruction pair

        rep_cm = tc.For_i(0, reps, 1)
        rep_cm.__enter__()

        # ---- setup (inside the rep loop so the rep-slope timing counts the
        #      full per-execution work): weights, biases, scratch init
        skip_setup = "setup" in ablate
        w1t = [wsp.tile([FCH, NFCH * FH], mdt, tag=f"w1t{k_}", name=f"w1t{k_}")
               for k_ in range(2)]
        w2t = [wsp.tile([128, NKB * F], mdt, tag=f"w2t{k_}", name=f"w2t{k_}")
               for k_ in range(2)]
        b1m = wsp.tile([128, NKB], fp, tag="b1m")
        b2m = wsp.tile([FCH, NFCH], fp, tag="b2m")
        if skip_setup:
            for k_ in range(2):
                nc.scalar.copy(w1t[k_][:, :1], ones[:FCH, :])
                nc.scalar.copy(w2t[k_][:, :1], ones[:, :])
            nc.vector.tensor_scalar(b1m[:], ones[:, :1], 0.0, None,
                                    op0=OP.mult)
            nc.vector.tensor_scalar(b2m[:], ones[:FCH, :1], 0.0, None,
                                    op0=OP.mult)
        else:
            for k_ in range(2):
                nc.sync.dma_start(
                    w1t[k_][:],
                    AP(w1_p, k_ * F * FH,
                       [[FH, FCH], [FCH * FH, NFCH], [1, FH]]))
                nc.sync.dma_start(
                    w2t[k_][:],
                    AP(w2_p, k_ * FHP * F,
                       [[F, 128], [128 * F, NKB], [1, F]]))
            nc.sync.dma_start(b1m[:], AP(b1_p, 0, [[1, 128], [128, NKB]]))
            nc.sync.dma_start(b2m[:], AP(b2_p, 0, [[1, FCH], [FCH, NFCH]]))
        for b in range(SPB):
            if skip_setup:
                continue
            ad, sd = a_scr[b], s_scr[b]
            nc.sync.dma_start(
                AP(ad, ABASE, [[1, 512 * WROW]]),
                AP(zeros_p, 0, [[0, 4], [1, 128 * WROW]]))
            nc.sync.dma_start(AP(sd, N, [[1, WROW - N]]),
                              AP(zeros_p, 0, [[1, 2 * (WROW - N)]]).bitcast(fp))
            # zero the skew-reload guard so group-0 garbage reads are zeros
            nc.sync.dma_start(AP(t_scr[b], 0, [[1, TBASE]]),
                              AP(zeros_p, 0, [[1, TBASE]]))

        # ================= phase 1: input loads + row sums =================
        xt_all, xs_all = [], []
        for b in range(SPB):
            sd = s_scr[b]
            xtm = xtp.tile([FCH, NFCH * WROW], mdt, tag=f"xtm{b}",
                           name=f"xtm{b}")
            if "phase1" in ablate:
                nc.scalar.copy(xtm[:, :1], ones[:FCH, :])
            else:
                nc.sync.dma_start(
                    xtm[:, :WROW],
                    AP(xpt_p, b * F * WROW, [[WROW, FCH], [1, WROW]]))
                nc.sync.dma_start(
                    xtm[:, WROW:],
                    AP(xpt_p, (b * F + FCH) * WROW,
                       [[WROW, FCH], [FCH * WROW, NFCH - 1], [1, WROW]]))
            xt = [xtm[:, c * WROW:(c + 1) * WROW] for c in range(NFCH)]
            xt_all.append(xt)
            # shifted copy: xs[f, i] = x[f, i+1]; odd band offsets read it at
            # an even element offset so the DVE max stays packed-mode
            xsm = xtp.tile([FCH, NFCH * WROW], mdt, tag=f"xsm{b}",
                           name=f"xsm{b}")
            if "phase1" in ablate:
                nc.scalar.copy(xsm[:, :1], ones[:FCH, :])
            else:
                nc.scalar.copy(xsm[:, :WROW], xtm[:, 1:WROW + 1])
                nc.scalar.copy(xsm[:, WROW:NFCH * WROW - 1],
                               xtm[:, WROW + 1:NFCH * WROW])
            xs_all.append([xsm[:, c * WROW:(c + 1) * WROW]
                           for c in range(NFCH)])
            psS = psb.tile([1, N], fp, tag="psS")
            for c in range(NFCH):
                nc.tensor.matmul(psS[:], ones[:FCH, :], xt[c][:, :N],
                                 start=(c == 0), stop=(c == NFCH - 1))
            srow = bnd.tile([1, N], fp, tag=f"srow{b}", name=f"srow{b}")
            nc.scalar.copy(srow[:], psS[:])
            nc.sync.dma_start(AP(sd, 0, [[1, N]]), srow[:])

        # ============ phase 2/3: bands and chebs, software-pipelined =======
        at_all = [None] * SPB
        dinv_all = [None] * SPB

        def gen_band(b):
            ad, sd, td = a_scr[b], s_scr[b], t_scr[b]
            xt, xs = xt_all[b], xs_all[b]
            psDc = psb.tile([1, N], fp, tag="psDc", name="psDc")
            first_deg = [True]
            d0 = 1
            while d0 <= w:
                dn = min(DCH, w - d0 + 1)
                psM = psb.tile([dn, N], fp, tag="psM", name="psM")
                nbatches = (dn + BD - 1) // BD
                mm_n = [0]
                mm_total = dn * NFCH
                for c in range(NFCH):
                    for bi in range(nbatches):
                        db0 = bi * BD
                        nb = min(BD, dn - db0)
                        offs = [d0 + db0 + j for j in range(nb)]
                        evens = [d for d in offs if d % 2 == 0]
                        odds = [d for d in offs if d % 2 == 1]
                        mxb = mxp.tile([FCH, BD * N], mdt, tag="mx",
                                       name="mxb")
                        base = xt[c][:, 0:N]
                        sbase = xs[c][:, 0:N]
                        slot_of = {}
                        slot = 0
                        for grp, src, shift in ((evens, base, 0),
                                                (odds, sbase, 1)):
                            if not grp:
                                continue
                            ng = len(grp)
                            in0 = bass.AP(base.tensor, base.offset,
                                          [list(base.ap[0]), [0, ng], [1, N]])
                            in1 = bass.AP(src.tensor,
                                          src.offset + grp[0] - shift,
                                          [list(src.ap[0]), [2, ng], [1, N]])
                            if "band_dve" not in ablate:
                                nc.vector.tensor_tensor(
                                    out=mxb[:, slot * N:(slot + ng) * N],
                                    in0=in0, in1=in1, op=OP.max)
                            else:
                                nc.vector.tensor_scalar(
                                    mxb[:, slot * N:(slot + ng) * N:512],
                                    xt[c][:, :ng], 1.0, None, op0=OP.mult)
                            for gi, d in enumerate(grp):
                                slot_of[d] = slot + gi
                            slot += ng
                        for d in offs:
                            di = d - d0
                            j = slot_of[d]
                            nc.tensor.matmul(
                                psM[:, :],
                                estep[:, 47 - di:47 - di + dn],
                                mxb[:, j * N:(j + 1) * N],
                                start=(mm_n[0] == 0),
                                stop=(mm_n[0] == mm_total - 1))
                            mm_n[0] += 1
                        yield
                # epilogue: D = 2M - S_i - S_{i+d}; masks -> strips
                sshift = bnd.tile([dn, N], fp, tag="sshift", name="sshift")
                nc.sync.dma_start(sshift[:], AP(sd, d0, [[1, dn], [1, N]]))
                sb_t = bnd.tile([dn, N], fp, tag="sb", name="sb_t")
                nc.sync.dma_start(sb_t[:], AP(sd, 0, [[0, dn], [1, N]]))
                ashift = bnd.tile([dn, N], fp, tag="ashift", name="ashift")
                nc.sync.dma_start(ashift[:],
                                  AP(attp_p, b * WROW + d0, [[1, dn], [1, N]]))
                ab_t = bnd.tile([dn, N], fp, tag="ab", name="ab_t")
                nc.sync.dma_start(ab_t[:],
                                  AP(attp_p, b * WROW, [[0, dn], [1, N]]))
                nc.vector.scalar_tensor_tensor(
                    out=sb_t[:], in0=sb_t[:], scalar=DIST_THRESH, in1=sshift[:],
                    op0=OP.add, op1=OP.add)
                nc.vector.scalar_tensor_tensor(
                    out=sshift[:], in0=psM[:], scalar=2.0, in1=sb_t[:],
                    op0=OP.mult, op1=OP.is_le)
                nc.vector.tensor_sub(ashift[:], ashift[:], ab_t[:])
                nc.vector.tensor_scalar(ab_t[:], ashift[:], ATT_THRESH, None,
                                        op0=OP.is_le)
                abnd = bnd.tile([dn, N], mdt, tag="abnd", name="abnd")
                nc.vector.tensor_mul(abnd[:], sshift[:], ab_t[:])
                # ---- strips -> per-node contiguous runs via PE transposes
                # upper triangle (+ diagonal ones when d0 == 1):
                #   run for node k covers cols [k+d0, k+d0+dn-1]
                # lower triangle: skewed reload V[dd,k] = abnd[dd, k-d0-dd],
                #   anti-transpose -> run covers cols [k-d0-dn+1, k-d0]
                Q = TBASE + d0
                nc.sync.dma_start(AP(td, Q, [[N, dn], [1, N]]), abnd[:])
                vsk = bnd.tile([dn, N], mdt, tag="vsk", name="vsk")
                nc.sync.dma_start(vsk[:],
                                  AP(td, TBASE, [[N - 1, dn], [1, N]]))
                # cols < d0 alias the previous strip's tail: zero them (their
                # scatter runs land in the guard/pad, but degree sums must
                # not see them)
                nc.vector.tensor_scalar(vsk[:, 0:d0], abnd[:, 0:d0], 0.0,
                                        None, op0=OP.mult)
                # degree contributions straight from the strips:
                # upper-deg[i] += sum_dd abnd[dd, i]; lower-deg[k] += sum_dd
                # vsk[dd, k]; +1 (diagonal) added on the reciprocal op
                nc.tensor.matmul(psDc[:], ones[:dn, :], abnd[:],
                                 start=first_deg[0], stop=False)
                first_deg[0] = False
                nc.tensor.matmul(psDc[:], ones[:dn, :], vsk[:],
                                 start=False, stop=(d0 + dn > w))
                if "scatter" in ablate:
                    d0 += dn
                    yield
                    continue
                utm = bnd.tile([128, 4 * 64], mdt, tag="utm", bufs=2,
                               name="utm")
                ltm = bnd.tile([128, 4 * 64], mdt, tag="ltm", bufs=2,
                               name="ltm")
                for t in range(4):
                    k0 = t * 128
                    psU = pst.tile([128, 128], mdt, tag="ps_t", name="psU")
                    nc.tensor.transpose(psU[:, :dn],
                                        abnd[:, k0:k0 + 128],
                                        ident[:dn, :dn])
                    if d0 == 1:
                        nc.scalar.copy(utm[:, t * 64:t * 64 + 1], ones[:, :])
                        nc.scalar.copy(utm[:, t * 64 + 1:t * 64 + 1 + dn],
                                       psU[:, :dn])
                    else:
                        nc.scalar.copy(utm[:, t * 64:t * 64 + dn],
                                       psU[:, :dn])
                    psL = pst.tile([128, 128], mdt, tag="ps_t", name="psL")
                    nc.tensor.transpose(psL[:, :dn],
                                        vsk[:, k0:k0 + 128],
                                        anti[dn][:dn, :dn])
                    nc.scalar.copy(ltm[:, t * 64:t * 64 + dn], psL[:, :dn])
                un = 1 + dn if d0 == 1 else dn
                uoff = 0 if d0 == 1 else d0
                nc.sync.dma_start(
                    AP(ad, ABASE + uoff,
                       [[WROW + 1, 128], [128 * (WROW + 1), 4], [1, un]]),
                    bass.AP(utm.tensor, utm.offset,
                            [list(utm.ap[0]), [64, 4], [1, un]]))
                nc.sync.dma_start(
                    AP(ad, ABASE - (d0 + dn - 1),
                       [[WROW + 1, 128], [128 * (WROW + 1), 4], [1, dn]]),
                    bass.AP(ltm.tensor, ltm.offset,
                            [list(ltm.ap[0]), [64, 4], [1, dn]]))
                d0 += dn
                yield

            # 1/deg row: deg = 1 (diag) + psDc; broadcast along partitions
            dinvr = amp.tile([1, N], mdt, tag=f"dinvr{b}", name=f"dinvr{b}")
            degr = bnd.tile([1, N], fp, tag="degr", name="degr")
            nc.vector.tensor_scalar(degr[:], psDc[:], 1.0, None, op0=OP.add)
            with nc.allow_low_precision(
                    reason="1/deg of integer degrees <=512: fp16 keeps "
                           "4.9e-4, well inside the 2e-2 budget"):
                nc.vector.reciprocal(dinvr[:], degr[:])
            dinv_all[b] = dinvr
            atm = amp.tile([128, 4 * N], mdt, tag=f"atm{b}", name=f"atm{b}")
            if "atm" in ablate:
                nc.scalar.copy(atm[:, :1], ones[:, :])
            else:
                nc.sync.dma_start(
                    atm[:],
                    AP(ad, ABASE, [[WROW, 128], [128 * WROW, 4], [1, N]]))
            # at' = (A+I) diag(1/deg): both Cheb terms use it directly.
            # Physically broadcast 1/deg across partitions with a 1-row
            # matmul (ones column x dinv row -> PSUM), then column-scale.
            psB = psp.tile([128, N], fp, tag="mm", name="psB")
            nc.tensor.matmul(psB[:], onesr[:, :], dinvr[:],
                             start=True, stop=True)
            din = bass.AP(psB.tensor, psB.offset,
                          [list(psB.ap[0]), [0, 4], [1, N]])
            nc.vector.tensor_tensor(out=atm[:], in0=atm[:], in1=din,
                                    op=OP.mult)
            at_all[b] = [atm[:, t * N:(t + 1) * N] for t in range(4)]
            yield

        def gen_cheb(b):
            if "cheb" in ablate:
                otm = otp.tile([FCH, NFCH * N], mdt, tag=f"otm{b}", name="otm")
                nc.scalar.copy(otm[:, :1], b2m[:, :1])
                nc.sync.dma_start(
                    AP(out_p, b * F * N,
                       [[N, FCH], [FCH * N, NFCH], [1, N]]),
                    otm[:])
                yield
                return
            xt, at = xt_all[b], at_all[b]
            xnm = xpp.tile([128, 4 * F], mdt, tag=f"xnm{b}", name="xnm")
            nc.sync.dma_start(
                xnm[:],
                AP(xp_p, b * N * F, [[F, 128], [128 * F, 4], [1, F]]))
            xn = [xnm[:, t * F:(t + 1) * F] for t in range(4)]
            yield

            zt = [acp.tile([FCH, N], mdt, tag=f"zt{b}{m}", name=f"zt{m}")
                  for m in range(NFCH)]
            for m in range(NFCH):
                psZ = psp.tile([FCH, N], fp, tag="mm", name="psZ")
                for t in range(4):
                    nc.tensor.matmul(psZ[:], xn[t][:, m * FCH:(m + 1) * FCH],
                                     at[t][:], start=(t == 0), stop=(t == 3))
                nc.scalar.copy(zt[m][:], psZ[:])
                yield

            ht = [acp.tile([128, N], mdt, tag=f"ht{b}{k}", name=f"ht{k}")
                  for k in range(NKB)]
            for k, (mo, mp_) in enumerate(FH_BLOCKS):
                psH = psp.tile([128, N], fp, tag="mm", name="psH")
                for c in range(NFCH):
                    nc.tensor.matmul(psH[:mp_, :],
                                     w1t[0][:, c * FH + mo:c * FH + mo + mp_],
                                     xt[c][:, :N], start=(c == 0), stop=False)
                for c in range(NFCH):
                    nc.tensor.matmul(psH[:mp_, :],
                                     w1t[1][:, c * FH + mo:c * FH + mo + mp_],
                                     zt[c][:], start=False,
                                     stop=(c == NFCH - 1))
                nc.scalar.activation(ht[k][:mp_, :], psH[:mp_, :], AF.Relu,
                                     bias=b1m[:mp_, k:k + 1], scale=1.0)
                yield

            qt = [acp.tile([128, N], mdt, tag=f"qt{b}{k}", name=f"qt{k}")
                  for k in range(NKB)]
            for k, (mo, mp_) in enumerate(FH_BLOCKS):
                psQ = psp.tile([128, N], fp, tag="mm", name="psQ")
                for t in range(4):
                    psT = pst.tile([128, 128], mdt, tag="ps_t", name="psT")
                    nc.tensor.transpose(
                        psT[:, :mp_],
                        ht[k][:mp_, t * 128:(t + 1) * 128],
                        ident[:mp_, :mp_])
                    hb = bnd.tile([128, 128], mdt, tag="hb", bufs=3,
                                  name="hb")
                    nc.scalar.copy(hb[:, :mp_], psT[:, :mp_])
                    nc.tensor.matmul(psQ[:mp_, :], hb[:, :mp_], at[t][:],
                                     start=(t == 0), stop=(t == 3))
                nc.scalar.copy(qt[k][:mp_, :], psQ[:mp_, :])
                yield

            otm = otp.tile([FCH, NFCH * N], mdt, tag=f"otm{b}", name="otm")
            for m in range(NFCH):
                psO = psp.tile([FCH, N], fp, tag="mm", name="psO")
                for k, (ko, kp) in enumerate(FH_BLOCKS):
                    nc.tensor.matmul(psO[:],
                                     w2t[0][:kp, k * F + m * FCH:
                                            k * F + (m + 1) * FCH],
                                     ht[k][:kp, :], start=(k == 0), stop=False)
                for k, (ko, kp) in enumerate(FH_BLOCKS):
                    nc.tensor.matmul(psO[:],
                                     w2t[1][:kp, k * F + m * FCH:
                                            k * F + (m + 1) * FCH],
                                     qt[k][:kp, :], start=False,
                                     stop=(k == NKB - 1))
                nc.scalar.activation(otm[:, m * N:(m + 1) * N], psO[:],
                                     AF.Relu, bias=b2m[:, m:m + 1], scale=1.0)
                yield
            nc.sync.dma_start(
                AP(out_p, b * F * N, [[N, FCH], [FCH * N, NFCH], [1, N]]),
                otm[:])

        # band0 fully; then interleave band1 units with cheb0 units so
        # ChebConv-0 matmuls fill PE slack while DVE paces band1; cheb1 last
        for _ in gen_band(0):
            pass
        g_band1, g_cheb0 = gen_band(1), gen_cheb(0)
        done_b = done_c = False
        while not done_b:
            for _ in range(2):
                if not done_b:
                    done_b = next(g_band1, StopIteration) is StopIteration
            if not done_c:
                done_c = next(g_cheb0, StopIteration) is StopIteration
        g_cheb1 = gen_cheb(1)
        done_c1 = False
        while not (done_c and done_c1):
            if not done_c:
                done_c = next(g_cheb0, StopIteration) is StopIteration
            if not done_c1:
                done_c1 = next(g_cheb1, StopIteration) is StopIteration

        rep_cm.__exit__(None, None, None)

    if not nc.is_finalized():
        nc.finalize()
    return nc, WROW


def _prepare(x4, attention):
    """Host prep: flatten, sort by attention, compute band width, pad."""
    X = np.ascontiguousarray(x4.reshape(B, N, F), dtype=np.float32)
    att = np.ascontiguousarray(attention[:, :, 0, 0], dtype=np.float32)
    perms = np.argsort(att, axis=1, kind="stable")
    attp = np.take_along_axis(att, perms, axis=1)
    a64 = attp.astype(np.float64)
    w = 1
    for bi in range(B):
        for d in range(1, N):
            if np.min(a64[bi, d:] - a64[bi, :-d]) <= ATT_THRESH + 1e-6:
                w = max(w, d)
            else:
                break  # windows only widen with d
    w = min(w, N - 1)
    Xp = np.take_along_axis(X, perms[:, :, None], axis=1)
    return Xp, attp, perms, w


def _make_runner(nc):
    """Compile the Bass program into a reusable 8-core sharded jax callable.

    Mirrors concourse.bass2jax.run_bass_via_pjrt's multi-core branch, but
    returns the compiled callable so repeated executions can be timed.
    """
    import jax
    from jax.sharding import Mesh, PartitionSpec
    from jax.experimental.shard_map import shard_map
    from concourse import bass2jax, mybir

    bass2jax.install_neuronx_cc_hook()

    in_names, out_names, out_avals, zero_outs = [], [], [], []
    partition_name = (nc.partition_id_tensor.name
                      if nc.partition_id_tensor else None)
    for alloc in nc.m.functions[0].allocations:
        if not isinstance(alloc, mybir.MemoryLocationSet):
            continue
        name = alloc.memorylocations[0].name
        if alloc.kind == "ExternalInput":
            if name != partition_name:
                in_names.append(name)
        elif alloc.kind == "ExternalOutput":
            shape = tuple(alloc.tensor_shape)
            dtype = mybir.dt.np(alloc.dtype)
            out_names.append(name)
            out_avals.append(jax.core.ShapedArray(shape, dtype))
            zero_outs.append(np.zeros(shape, dtype))
    n_params = len(in_names)
    n_outs = len(out_avals)
    in_names = in_names + out_names
    if partition_name is not None:
        in_names.append(partition_name)
    donate = tuple(range(n_params, n_params + n_outs))

    def _body(*args):
        operands = list(args)
        if partition_name is not None:
            operands.append(bass2jax.partition_id_tensor())
        outs = bass2jax._bass_exec_p.bind(
            *operands,
            out_avals=tuple(out_avals),
            in_names=tuple(in_names),
            out_names=tuple(out_names),
            lowering_input_output_aliases=(),
            sim_require_finite=True,
            sim_require_nnan=True,
            nc=nc,
        )
        return tuple(outs)

    devices = jax.devices()[:NCORES]
    mesh = Mesh(np.asarray(devices), ("core",))
    sharded = jax.jit(
        shard_map(_body, mesh=mesh,
                  in_specs=(PartitionSpec("core"),) * (n_params + n_outs),
                  out_specs=(PartitionSpec("core"),) * n_outs,
                  check_rep=False),
        donate_argnums=donate, keep_unused=True)

    param_order = in_names[:n_params]

    def run(in_maps):
        concat_in = [
            np.concatenate([np.asarray(in_maps[c][nm]) for c in range(NCORES)],
                           axis=0)
            for nm in param_order
        ]
        concat_zeros = [np.zeros((NCORES * z.shape[0], *z.shape[1:]), z.dtype)
                        for z in zero_outs]
        out_arrs = jax.block_until_ready(sharded(*concat_in, *concat_zeros))
        return [
            {nm: np.asarray(out_arrs[i]).reshape(NCORES, *out_avals[i].shape)[c]
             for i, nm in enumerate(out_names)}
            for c in range(NCORES)
        ]

    return {"run": run, "sharded": sharded, "param_order": param_order,
            "zero_outs": zero_outs, "out_names": out_names,
            "out_avals": out_avals, "mesh": mesh}


def _get_runner(w):
    import os
    reps = int(os.environ.get("KERNEL_REPS", "1"))
    ablate = tuple(a for a in os.environ.get("KERNEL_ABLATE", "").split(",")
                   if a)
    key = (w, reps, ablate)
    if key not in _prog_cache:
        nc, WROW = _build_program(w, reps=reps, ablate=ablate)
        _prog_cache[key] = (_make_runner(nc), WROW)
    return _prog_cache[key]


def kernel(x4, attention, W1, b1, W2, b2):
    Xp, attp, perms, w = _prepare(x4, attention)
    runner, WROW = _get_runner(w)

    Xp16 = Xp.astype(np.float16)
    xpt = np.zeros((B, F, WROW), np.float16)
    xpt[:, :, :N] = Xp16.transpose(0, 2, 1)
    attp_pad = np.full((B, WROW), 1e9, np.float32)
    attp_pad[:, :N] = attp

    W1h = np.asarray(W1, dtype=np.float16)
    W2h = np.zeros((2, FHP, F), np.float16)
    W2h[:, :FH, :] = np.asarray(W2, dtype=np.float16)
    b1p = np.zeros((FHP,), np.float32)
    b1p[:FH] = np.asarray(b1, dtype=np.float32)
    b2p = np.zeros((F + 15,), np.float32)
    b2p[:F] = np.asarray(b2, dtype=np.float32)

    c_ones = np.ones((128, 1), np.float16)
    c_onesr = np.ones((1, 128), np.float16)
    c_estep = np.zeros((FCH, 95), np.float16)
    c_estep[:, 47] = 1.0
    c_ident = np.eye(128, dtype=np.float16)
    dn_sizes = sorted({min(DCH, w - d0 + 1) for d0 in range(1, w + 1, DCH)})
    c_anti = np.zeros((len(dn_sizes), DCH, DCH), np.float16)
    for si, dsz in enumerate(dn_sizes):
        c_anti[si, np.arange(dsz), dsz - 1 - np.arange(dsz)] = 1.0
    c_zeros = np.zeros((128, WROW), np.float16)

    in_maps = []
    for c in range(NCORES):
        sl = slice(c * SPB, (c + 1) * SPB)
        in_maps.append({
            "xpt": np.ascontiguousarray(xpt[sl]),
            "xp": np.ascontiguousarray(Xp16[sl]),
            "attp": np.ascontiguousarray(attp_pad[sl]),
            "w1": W1h, "b1": b1p, "w2": W2h, "b2": b2p,
            "c_ones": c_ones, "c_onesr": c_onesr,
            "c_estep": c_estep, "c_ident": c_ident,
            "c_anti": c_anti, "c_zeros": c_zeros,
        })

    # The first execution of a freshly-compiled NEFF occasionally wedges a
    # core (NRT_EXEC_UNIT_UNRECOVERABLE); the device recovers on a fresh
    # PJRT client.  Retry with a rebuilt runner before giving up.
    for attempt in range(3):
        try:
            results = runner["run"](in_maps)
            break
        except Exception:
            if attempt == 2:
                raise
            import jax, time as _time
            _time.sleep(2.0)
            try:
                jax.clear_backends()
            except Exception:
                pass
            _prog_cache.clear()
            runner, WROW = _get_runner(w)
    globals()["last_in_maps"] = in_maps
    globals()["last_runner"] = runner

    inv = np.argsort(perms, axis=1)
    out = np.empty((B, N, F), np.float32)
    for c in range(NCORES):
        o = results[c]["outT"]  # [SPB, F, N] (fp16 on device)
        for s in range(SPB):
            bi = c * SPB + s
            out[bi] = o[s].T.astype(np.float32)[inv[bi]]
    return out
